# revision 64
# baseline (speedup 1.0000x reference)
"""Trainium2 Bass kernel for nn_Attention_45724221833663 (sparse_attention).

Strategy: data-parallel over batch B=8 across the 8 NeuronCores (one batch
element per core).  Matmul dtype mix: the q/k projections run in fp8e4
DoubleRow (2 contraction rows per PE cell -> half the matmul passes), the
softmax weights E are stored fp8 (AV = fp8 stationary x bf16 moving), and
everything whose error would reach the output directly (v projection,
scores operands, AV values, output projection) stays bf16 with fp32 PSUM.
Validated end-to-end: amax rel err ~8e-3 vs fp32 reference (tol 2e-2).

Per-core dataflow (host pre-transposes weights/x while sharding; ln_g is
folded into Wp, ln_b into bp):
  xcat8  [c=1024, kvp=1152] fp8  (concat(x_text,x).T, q/k weights x256 on
         host, the 1/65536 is folded into the exp scale)
  xcatT  [c, kvp] bf16           (v projection only)
  vw     [kvp, h, 65] bf16 = (xcatT.T @ WvT) per head + ones column
  qT/kT via DoubleRow fp8: 4 passes of K=256 instead of 8 of K=128
  per head pair (even head on PE row-tile 0, odd on row-tile 64):
    scores for both heads go into ONE [128,1024] psum tile per (kv-tile,
    n-half): disjoint PE row groups + disjoint psum banks -> the two
    matmuls execute concurrently.
    E = exp(scoresT/(8*65536)) -> fp8 e2 tile; kv=0 row and pad rows zeroed
    avp[n,0:65] = sum_kv E[kv,..] @ vw[kv,h,:]  (col 64 = S[n])
    attn[n, h*64:+64] = avp[:, :64]*(1/S) + tanh(g_h)*v_h[kv=0]
  Final head pair runs n-tile-major with LN stats + output projection
  interleaved.  The projection consumes RAW attn (transposed via PE), and
  LayerNorm is applied algebraically at psum evacuation:
      out = (attnT @ Wpf) * rstd - (mu*rstd*colsum(Wpf) - bp)
  so the PE transpose/matmul chain never waits on the LN stats.  rstd is
  computed as exp(-0.5*ln(var+eps)) so ScalarE stays on the
  natural_log_exp table set (no mid-kernel table switch).  Transposes are
  hoisted 2 chunks ahead of their matmuls and their psum->sbuf copies run
  on ScalarE (idle there: all exps are done by that phase).  Output is
  written bf16 and upcast on the host.
"""

import os
import numpy as np
import ml_dtypes

import concourse.bacc as bacc
import concourse.tile as tile
from concourse import mybir
from concourse.masks import make_identity
from concourse.bass_utils import run_bass_kernel_spmd

F32 = mybir.dt.float32
BF16 = mybir.dt.bfloat16
E4 = mybir.dt.float8e4
AF = mybir.ActivationFunctionType
OP = mybir.AluOpType
DRMODE = mybir.MatmulPerfMode.DoubleRow

B, N, P, DIM, H = 8, 1024, 77, 1024, 16
HD = DIM // H          # 64
KV = P + N             # 1101
KT = 9                 # kv tiles of 128
KVP = KT * 128         # 1152 padded
NT = N // 128          # 8 n tiles
CC = DIM // 128        # 8 contraction chunks
DR = CC // 2           # 4 double-row chunks (K=256 each)
OT = DIM // 128        # 8 output-channel tiles
LN_EPS = 1e-5
WSCALE = 256.0         # host premultiply on Wq/Wk so fp8 sees ~N(0,5) values
EXP_SCALE = 0.125 / (WSCALE * WSCALE)

LAST_EXEC_NS = None
_CACHE = {}


def _emit(tc, with_bias=False):
    nc = tc.nc

    xcat8_d = nc.dram_tensor("xcat8", [DIM, KVP], E4, kind="ExternalInput").ap()
    xcatT_d = nc.dram_tensor("xcatT", [DIM, KVP], BF16, kind="ExternalInput").ap()
    wq8_d = nc.dram_tensor("wq8T", [DIM, DIM], E4, kind="ExternalInput").ap()
    wk8_d = nc.dram_tensor("wk8T", [DIM, DIM], E4, kind="ExternalInput").ap()
    wv_d = nc.dram_tensor("wvT", [DIM, DIM], BF16, kind="ExternalInput").ap()
    wp_d = nc.dram_tensor("wpT", [DIM, DIM], BF16, kind="ExternalInput").ap()
    tanhg_d = nc.dram_tensor("tanhg", [1, H], F32, kind="ExternalInput").ap()
    wbarbp_d = nc.dram_tensor("wbarbp", [2, DIM], BF16, kind="ExternalInput").ap()
    out_d = nc.dram_tensor("out", [N, DIM], BF16, kind="ExternalOutput").ap()

    xcat8_re = xcat8_d.rearrange("(j p) f -> p j f", p=128)
    xcat_re = xcatT_d.rearrange("(j p) f -> p j f", p=128)
    wq8_re = wq8_d.rearrange("(j p) o -> p j o", p=128)
    wk8_re = wk8_d.rearrange("(j p) o -> p j o", p=128)
    wv_re = wv_d.rearrange("(j p) o -> p j o", p=128)
    wp_re = wp_d.rearrange("(j p) o -> p j o", p=128)

    from contextlib import ExitStack

    with ExitStack() as top:
        consts = top.enter_context(tc.tile_pool(name="consts", bufs=1))
        acts = top.enter_context(tc.tile_pool(name="acts", bufs=1))
        ph1 = top.enter_context(tc.tile_pool(name="ph1", bufs=1))
        wstream = top.enter_context(tc.tile_pool(name="wstream", bufs=8))
        qkp = top.enter_context(tc.tile_pool(name="qkp", bufs=3))
        epool = top.enter_context(tc.tile_pool(name="epool", bufs=4))
        tpool = top.enter_context(tc.tile_pool(name="tmp", bufs=4))
        opool = top.enter_context(tc.tile_pool(name="outp", bufs=4))
        t1p = top.enter_context(tc.tile_pool(name="t1p", bufs=2))
        ps_proj = top.enter_context(tc.tile_pool(name="ps_proj", bufs=2, space="PSUM"))
        ps_scores = top.enter_context(
            tc.tile_pool(name="ps_scores", bufs=1, space="PSUM"))
        ps_av = top.enter_context(tc.tile_pool(name="ps_av", bufs=2, space="PSUM"))

        # ---- constants ----
        tanhg_sb = consts.tile([128, H], F32, tag="tanhg")
        eps_t = consts.tile([128, 1], F32, tag="eps")
        nc.vector.memset(eps_t, LN_EPS)
        ident = consts.tile([128, 128], BF16, tag="ident")
        make_identity(nc, ident)
        wbar_b = consts.tile([128, DIM], BF16, tag="wbar")
        bp_b = consts.tile([128, DIM], BF16, tag="bpb")

        # p-state warmup: the PE idles ~5us waiting for the first input
        # chunks and then runs pair 0's projections at the cold 1.2GHz
        # p-state (it only reaches 2.4GHz after ~3.4us of continuous work).
        # Burn that idle window on zero matmuls sized to drain just before
        # the real ones are data-ready.
        warm = consts.tile([128, 512], BF16, tag="warm")
        nc.vector.memset(warm, 0.0)
        wps = ps_av.tile([128, 512], F32, tag="avp")
        for i in range(12):
            nc.tensor.matmul(wps, warm[:, 0:128], warm,
                             start=(i == 0), stop=(i == 11))

        def emit_fillers(n):
            # zero matmuls emitted AHEAD of a DMA-gated projection: they
            # execute while the projection's inputs are still in flight, so
            # the PE stays busy and HAM never re-throttles the clock.  Cost
            # if the data was actually ready: n x ~216ns.
            for i in range(n):
                nc.tensor.matmul(wps, warm[:, 0:128], warm,
                                 start=(i == 0), stop=(i == n - 1))

        # ---- persistent activations ----
        vw_sb = acts.tile([128, KT, H, HD + 1], BF16, tag="vw")  # [kv-part, kv-tile, h, d+1]
        attn_sb = acts.tile([128, NT, H, HD], BF16, tag="attn")  # [n-part, n-tile, h, d]
        # transposed attn chunks for the output projection.  Chunk cc of the
        # channel dim is exactly head pair cc, so each pair's transposes run
        # right after its AV tails -- spread across the whole kernel instead
        # of bunched into the final phase.
        lt_sb = acts.tile([128, NT, CC, 128], BF16, tag="lt")    # [c-part, n-tile, pair, 2*d]

        # input loads, c-chunk granular; fp8 xcat gates the q/k projections
        # (and hence scores + the exp stream) so it loads first; the bf16
        # xcat only feeds the v projection ~25us in.
        xcat8_sb = ph1.tile([128, CC, KVP], E4, tag="xcat8")
        xcatT_sb = ph1.tile([128, CC, KVP], BF16, tag="xcatT")
        # wv and wp share one slot: wv dies after the v projection, wp is
        # only needed from the output projection onwards
        wv_sb = ph1.tile([128, CC, DIM], BF16, tag="wvwp")
        # descriptor issue on an engine queue costs ~600ns each; spread the
        # startup-critical loads over the three DMA-capable queues.  Weight
        # streams for upcoming o-tiles are prefetched 2-3 pairs ahead so a
        # q/k projection never waits behind bulk xcat/wv traffic.
        w0q = wstream.tile([128, CC, 128], E4, tag="w")
        nc.scalar.dma_start(out=w0q, in_=wq8_re[:, :, 0:128])
        w0k = wstream.tile([128, CC, 128], E4, tag="w")
        nc.scalar.dma_start(out=w0k, in_=wk8_re[:, :, 0:128])

        wtiles = {}

        def prefetch_w(ot, q):
            if ot >= OT or ot in wtiles:
                return
            wq_t = wstream.tile([128, CC, 128], E4, tag="w", name=f"wq{ot}")
            q.dma_start(out=wq_t, in_=wq8_re[:, :, ot * 128:(ot + 1) * 128])
            wk_t = wstream.tile([128, CC, 128], E4, tag="w", name=f"wk{ot}")
            q.dma_start(out=wk_t, in_=wk8_re[:, :, ot * 128:(ot + 1) * 128])
            wtiles[ot] = (wq_t, wk_t)

        xcat_q = [nc.gpsimd, nc.gpsimd, nc.gpsimd, nc.gpsimd,
                  nc.scalar, nc.sync, nc.sync, nc.sync]
        # two pieces per chunk, all first pieces ahead of all second pieces:
        # cols 0:640 cover everything the q half-0 (cols 77:589) and the
        # k split-0 (cols 0:512) projections read, so their matmuls -- and
        # the first scores/exps -- start sooner than with whole-chunk loads
        for cc in range(CC):
            xcat_q[cc].dma_start(out=xcat8_sb[:, cc, 0:640],
                                 in_=xcat8_re[:, cc, 0:640])
        for cc in range(CC):
            xcat_q[cc].dma_start(out=xcat8_sb[:, cc, 640:KVP],
                                 in_=xcat8_re[:, cc, 640:KVP])
        # weights for pairs 1-3 on the (light) scalar ring, behind only the
        # two startup-critical xcat8 pieces it carries
        for ot in (1, 2, 3):
            prefetch_w(ot, nc.scalar)
        # bf16 xcat for the v projection: after the fp8 criticals
        xcatT_q = [nc.gpsimd, nc.gpsimd, nc.gpsimd, nc.gpsimd,
                   nc.sync, nc.sync, nc.sync, nc.sync]
        for cc in range(CC):
            xcatT_q[cc].dma_start(out=xcatT_sb[:, cc, :], in_=xcat_re[:, cc, :])

        # ---- q/k projections interleaved with their dependent head pairs,
        # so ScalarE (exp) fills while PE still runs projections ----
        last_rows = KV - (KT - 1) * 128  # 77

        def vproj_groups():
            # v projection into vw (head-interleaved), natural [kv, o]
            # layout, as 18 independent closures for interleaving
            def grp(kvt, half):
                def run():
                    ps = ps_proj.tile([128, 512], F32, tag="ps")
                    for cc in range(CC):
                        nc.tensor.matmul(
                            ps,
                            xcatT_sb[:, cc, kvt * 128:(kvt + 1) * 128],
                            wv_sb[:, cc, half * 512:(half + 1) * 512],
                            start=(cc == 0),
                            stop=(cc == CC - 1),
                        )
                    nc.vector.tensor_copy(
                        vw_sb[:, kvt, half * 8:(half + 1) * 8, 0:HD],
                        ps.rearrange("p (h d) -> p h d", d=HD),
                    )
                return run
            return [grp(kvt, half) for kvt in range(KT) for half in range(2)]

        def emit_qk_groups(ot, wtq=None, wtk=None):
            """q/k projections for o-tile ot as 5 closures (2 q halves,
            2 k splits, k text tail) so they can interleave between the
            scores chunks of the previous pair."""
            qt = qkp.tile([128, N], BF16, tag="qt")
            kt = qkp.tile([128, KVP], BF16, tag="kt")
            # pad keys (kv 1101:1152) are zero; scores psum partitions for
            # them are never read by the exp, but zero them for the checker
            nc.gpsimd.memset(kt[:, KV:KVP], 0.0)
            if wtq is None and ot in wtiles:
                wtq, wtk = wtiles.pop(ot)
            prefetch_w(ot + 2, nc.gpsimd)
            if wtq is None:
                wtq = wstream.tile([128, CC, 128], E4, tag="w")
                nc.sync.dma_start(out=wtq, in_=wq8_re[:, :, ot * 128:(ot + 1) * 128])
            if wtk is None:
                wtk = wstream.tile([128, CC, 128], E4, tag="w")
                nc.sync.dma_start(out=wtk, in_=wk8_re[:, :, ot * 128:(ot + 1) * 128])

            def qhalf(half):
                def run():
                    ps = ps_proj.tile([128, 512], F32, tag="ps")
                    for j in range(DR):
                        nc.tensor.matmul(
                            ps,
                            wtq[:, 2 * j:2 * j + 2, :],
                            xcat8_sb[:, 2 * j:2 * j + 2,
                                     P + half * 512: P + (half + 1) * 512],
                            start=(j == 0),
                            stop=(j == DR - 1),
                            perf_mode=DRMODE,
                        )
                    nc.vector.tensor_copy(qt[:, half * 512:(half + 1) * 512], ps)
                return run

            def ksplit(off):
                def run():
                    ps = ps_proj.tile([128, 512], F32, tag="ps")
                    for j in range(DR):
                        nc.tensor.matmul(
                            ps,
                            wtk[:, 2 * j:2 * j + 2, :],
                            xcat8_sb[:, 2 * j:2 * j + 2, off:off + 512],
                            start=(j == 0),
                            stop=(j == DR - 1),
                            perf_mode=DRMODE,
                        )
                    nc.vector.tensor_copy(kt[:, off:off + 512], ps)
                return run

            def k77():
                # 77-col text tail: DoubleRow loses below 128 free cols, so
                # run it as plain fp8 matmuls (bf16 rate)
                ps = ps_proj.tile([128, 512], F32, tag="ps")
                for cc in range(CC):
                    nc.tensor.matmul(
                        ps[:, :last_rows],
                        wtk[:, cc, :],
                        xcat8_sb[:, cc, 1024:KV],
                        start=(cc == 0),
                        stop=(cc == CC - 1),
                    )
                nc.vector.tensor_copy(kt[:, 1024:KV], ps[:, :last_rows])

            return qt, kt, [qhalf(0), ksplit(0), qhalf(1), ksplit(512), k77]

        def new_e2():
            # E layout: [kv-part, kv-tile, n-half, head, 512], fp8
            e2 = epool.tile([128, KT, 2, 2, 512], E4, tag="e")
            nc.gpsimd.memset(e2[:, KT - 1], 0.0)
            return e2

        def emit_scores_chunk(qt, kt, e2, kvt):
            # Scores for one kv-tile of the pair: all four (n-half, head)
            # quadrants land in ONE [128,2048] psum tile (4 banks), drained
            # by ONE 2048-col exp -- the ScalarE ~293ns/instruction fixed
            # cost amortizes over twice the columns vs per-half acts.  The
            # single psum slot means the NEXT chunk's matmuls wait for this
            # exp, so chunks must be interleaved with AV/proj group work.
            # Head pairs on disjoint PE row groups still run concurrently.
            rows = last_rows if kvt == KT - 1 else 128
            ps = ps_scores.tile([128, 2048], F32, tag="pss")
            for half in range(2):
                base = half * 1024
                nc.tensor.matmul(
                    ps[:, base:base + 512],
                    kt[0:64, kvt * 128:(kvt + 1) * 128],
                    qt[0:64, half * 512:(half + 1) * 512],
                    start=True, stop=True,
                )
                nc.tensor.matmul(
                    ps[:, base + 512:base + 1024],
                    kt[64:128, kvt * 128:(kvt + 1) * 128],
                    qt[64:128, half * 512:(half + 1) * 512],
                    start=True, stop=True,
                )
            nc.scalar.activation(
                e2[:rows, kvt].rearrange("p a b c -> p (a b c)"),
                ps[:rows], AF.Exp, bias=0.0, scale=EXP_SCALE)
            if kvt == 0:
                # first key column is gated separately
                nc.gpsimd.memset(e2[0:1, 0], 0.0)

        def eslice(e2, hh, kvt, nt):
            q, r = divmod(nt, 4)
            return e2[:, kvt, q, hh, r * 128:(r + 1) * 128]

        def emit_gv0(h):
            gv0 = tpool.tile([128, HD], BF16, tag="gv0")
            nc.gpsimd.partition_broadcast(gv0, vw_sb[0:1, 0, h, 0:HD])
            gv0s = tpool.tile([128, HD], F32, tag="gv0s")
            nc.vector.tensor_scalar_mul(gv0s, gv0, tanhg_sb[:, h:h + 1])
            return gv0s

        def emit_tr(p, nt, engine=None):
            """Transpose attn heads (2p, 2p+1) at n-tile nt into LT --
            chunk p of the output projection's lhsT.  Interleaved into the
            pair's AV loop, spreading the PE transposes and their psum
            evacuations (DVE mid-kernel -- gpsimd cannot read PSUM on trn2;
            ScalarE in the final phase where the exp stream is done)."""
            pst = ps_av.tile([128, 128], BF16, tag="avp")
            nc.tensor.transpose(
                pst,
                attn_sb[:, nt, 2 * p:2 * p + 2, :].rearrange("p h d -> p (h d)"),
                ident,
            )
            if engine is None:
                nc.vector.tensor_copy(lt_sb[:, nt, p, :], pst)
            else:
                engine.copy(out=lt_sb[:, nt, p, :], in_=pst)

        def emit_av_nt(p, e2, nt, gv0s_e, gv0s_o, alt, tr_p=None):
            """AV + fixup for BOTH heads of pair p at n-tile nt, plus the
            LT transpose of pair tr_p (lagged one pair behind, so its attn
            input's DVE fixups are long done and the PE never waits).
            Both heads accumulate into one [128,2,65] psum tile (one
            bank), so the pair costs one psum slot turn and one reciprocal
            instead of two -- the DVE's per-instruction fixed cost was the
            hidden hog at [128,64] granularity."""
            if alt and nt % 2 == 1:
                avp = ps_proj.tile([128, 2, HD + 1], F32, tag="ps")
            else:
                avp = ps_av.tile([128, 2, HD + 1], F32, tag="avp")
            for hh in range(2):
                for kvt in range(KT):
                    nc.tensor.matmul(
                        avp[:, hh, :],
                        eslice(e2, hh, kvt, nt),
                        vw_sb[:, kvt, 2 * p + hh, :],
                        start=(kvt == 0),
                        stop=(kvt == KT - 1),
                    )
            rs2 = tpool.tile([128, 2], F32, tag="rs")
            nc.vector.reciprocal(rs2, avp[:, :, HD])
            nc.vector.scalar_tensor_tensor(
                out=attn_sb[:, nt, 2 * p, :],
                in0=avp[:, 0, 0:HD],
                scalar=rs2[:, 0:1],
                in1=gv0s_e,
                op0=OP.mult,
                op1=OP.add,
            )
            nc.vector.scalar_tensor_tensor(
                out=attn_sb[:, nt, 2 * p + 1, :],
                in0=avp[:, 1, 0:HD],
                scalar=rs2[:, 1:2],
                in1=gv0s_o,
                op0=OP.mult,
                op1=OP.add,
            )
            if tr_p is not None:
                emit_tr(tr_p, nt)

        def emit_pair_tail(p, e2, alt=False, also_self_tr=False):
            """Everything after E for pair p: gate prep, AV + fixup, plus
            the LAGGED LT transposes of pair p-1 (and p's own when this is
            the last non-final tail).  alt=True additionally cycles the
            (by-then idle) proj psum pool for deeper AV pipelining."""
            gv0s_e = emit_gv0(2 * p)
            gv0s_o = emit_gv0(2 * p + 1)
            for nt in range(NT):
                emit_av_nt(p, e2, nt, gv0s_e, gv0s_o, alt,
                           tr_p=(p - 1 if p >= 1 else None))
                if also_self_tr:
                    emit_tr(p, nt)

        # ---- LN stats per n-tile.  Only mu/rstd are computed here; the
        # normalization itself is folded into the projection's psum
        # evacuation, so the PE transposes/matmuls never wait on it. ----
        def emit_stats(nt):
            xa = attn_sb[:, nt].rearrange("p h d -> p (h d)")
            xs = xa.rearrange("p (s f) -> p s f", f=512)
            stats = tpool.tile([128, 2, 6], F32, tag="stats")
            for s in range(2):
                nc.vector.bn_stats(stats[:, s, :], xs[:, s, :])
            mv = tpool.tile([128, 2], F32, tag="mv")
            nc.vector.bn_aggr(mv, stats)
            # sqrt + DVE reciprocal.  All sqrts happen after the last exp,
            # so ScalarE switches table sets exactly once (a dummy sqrt
            # right after the last scores pair pre-pays the ~1.3us load);
            # Copy lives in every set, so the psum evacuations never force
            # a reload.  (Ln/Exp rstd alternated table sets PER N-TILE --
            # ~1.3us reload each way, caught in the trace.)
            rstd = tpool.tile([128, 1], F32, tag="rstd")
            nc.scalar.activation(rstd, mv[:, 1:2], AF.Sqrt, bias=eps_t, scale=1.0)
            nc.vector.reciprocal(rstd, rstd)
            m2 = tpool.tile([128, 1], F32, tag="m2")
            nc.vector.tensor_scalar_mul(m2, mv[:, 0:1], rstd)
            # t1[n, o] = mu*rstd*colsum(Wpf)[o] (- bp[o] when the folded
            # bias is nonzero; it is zero here so the fast path uses a
            # cheap 2-operand multiply)
            t1 = t1p.tile([128, DIM], BF16, tag="t1")
            if with_bias:
                nc.vector.scalar_tensor_tensor(
                    out=t1, in0=wbar_b, scalar=m2, in1=bp_b,
                    op0=OP.mult, op1=OP.subtract,
                )
            else:
                nc.vector.tensor_scalar_mul(t1, wbar_b, m2)
            return rstd, t1

        # pair 0's scores/exp are hoisted before the v projection so ScalarE
        # starts as early as possible
        # software pipeline: scores/exp run one head-pair ahead of the
        # AV/fixup tails so ScalarE never starves
        # ---- main flow.  kvt-major scores: each chunk's 2048-col exp owns
        # the single 4-bank scores psum, so chunks are interleaved with
        # ~2us of AV/projection group work apiece. ----
        qt_c, kt_c, g0 = emit_qk_groups(0, w0q, w0k)
        for g in g0:
            emit_fillers(2)
            g()
        # wv split sync/scalar: one queue alone delivers the last chunk too
        # late for the v-projection (keep gpsimd free for the e2 memsets;
        # the scalar queue's exp stream only starts at the first pair)
        for cc in range(CC):
            dmae = nc.sync if cc % 2 == 0 else nc.scalar
            dmae.dma_start(out=wv_sb[:, cc, :], in_=wv_re[:, cc, :])
        # ones column for the row-sum S (E rows for kv=0/pad are zeroed);
        # disjoint from the v-projection's columns, so set it up front
        nc.gpsimd.memset(vw_sb[:, :, :, HD:HD + 1], 1.0)
        # tanh(gate): first consumer is the head tails ~45us in, keep it off
        # the startup-critical queues
        nc.sync.dma_start(out=tanhg_sb, in_=tanhg_d.to_broadcast([128, H]))

        vgs = vproj_groups()
        pend = []
        # period 0: scores pair 0 + qk pair 1 + fillers (DMA-gated window)
        e2a = new_e2()
        qt_n, kt_n, gq = emit_qk_groups(1)
        for kvt in range(KT):
            emit_fillers(2)
            if gq:
                gq.pop(0)()
            emit_scores_chunk(qt_c, kt_c, e2a, kvt)
        pend.append(e2a)
        qt_c, kt_c = qt_n, kt_n
        # period 1: scores 1 + qk 2 + first v-projection groups
        e2a = new_e2()
        qt_n, kt_n, gq = emit_qk_groups(2)
        for kvt in range(KT):
            if gq:
                gq.pop(0)()
            elif vgs:
                vgs.pop(0)()
            emit_scores_chunk(qt_c, kt_c, e2a, kvt)
        pend.append(e2a)
        qt_c, kt_c = qt_n, kt_n
        # period 2: scores 2 + qk 3 + v projection (the rest trails, before
        # any AV tail needs vw)
        e2a = new_e2()
        qt_n, kt_n, gq = emit_qk_groups(3)
        for kvt in range(KT):
            if gq:
                gq.pop(0)()
            if vgs:
                vgs.pop(0)()
            emit_scores_chunk(qt_c, kt_c, e2a, kvt)
        pend.append(e2a)
        qt_c, kt_c = qt_n, kt_n
        for g in vgs:
            g()
        wp_sb = ph1.tile([128, CC, DIM], BF16, tag="wvwp")
        for cc in range(CC):
            dmae = nc.gpsimd if cc < 4 else nc.sync
            dmae.dma_start(out=wp_sb[:, cc, :], in_=wp_re[:, cc, :])
        # folded colsum(Wpf)/bp broadcasts: consumed from ~200us
        nc.sync.dma_start(out=wbar_b, in_=wbarbp_d[0:1, :].to_broadcast([128, DIM]))
        nc.sync.dma_start(out=bp_b, in_=wbarbp_d[1:2, :].to_broadcast([128, DIM]))
        # periods 3-7: tails of pair k-3 + scores of pair k + qk of k+1
        for k in range(3, OT):
            p = k - 3
            e2a = new_e2()
            if k + 1 < OT:
                qt_n, kt_n, gq = emit_qk_groups(k + 1)
            else:
                qt_n = kt_n = None
                gq = []
            ep = pend.pop(0)
            gv0s_e = emit_gv0(2 * p)
            gv0s_o = emit_gv0(2 * p + 1)
            for nt in range(NT):
                emit_av_nt(p, ep, nt, gv0s_e, gv0s_o, alt=False,
                           tr_p=(p - 1 if p >= 1 else None))
                if gq:
                    gq.pop(0)()
                emit_scores_chunk(qt_c, kt_c, e2a, nt)
            emit_scores_chunk(qt_c, kt_c, e2a, KT - 1)
            pend.append(e2a)
            qt_c, kt_c = qt_n, kt_n
            done = p + 1

        def emit_outproj(nt, rstd, t1):
            # project the transposed raw-attn chunks (banked in LT across
            # the whole kernel): out[n, o] = (attn @ Wpf.T)*rstd - t1
            # pp psum alternates between the proj pool and the (dead by
            # now) scores pool, giving a 4-slot rotation: the projection of
            # nt+1 never WAR-waits on nt's psum evacuation.
            if nt % 2 == 0:
                pp0 = ps_proj.tile([128, 512], F32, tag="ps")
                pp1 = ps_proj.tile([128, 512], F32, tag="ps")
            else:
                pp = ps_scores.tile([128, 2, 512], F32, tag="pss")
                pp0, pp1 = pp[:, 0], pp[:, 1]
            for cc in range(CC):
                nc.tensor.matmul(
                    pp0, lt_sb[:, nt, cc, :], wp_sb[:, cc, 0:512],
                    start=(cc == 0), stop=(cc == CC - 1),
                )
                nc.tensor.matmul(
                    pp1, lt_sb[:, nt, cc, :], wp_sb[:, cc, 512:1024],
                    start=(cc == 0), stop=(cc == CC - 1),
                )
            # evacuation with the rstd scale fused into the ScalarE psum
            # copy (free: the scale slot of ACTIVATE Copy takes a
            # per-partition AP); ScalarE is idle in this phase once the exp
            # backlog drains, while the DVE carries stats + fixups.  The
            # mean correction is then a cheap 2-operand subtract.
            or0 = opool.tile([128, 512], BF16, tag="or")
            or1 = opool.tile([128, 512], BF16, tag="or")
            nc.scalar.activation(or0, pp0, AF.Copy, bias=0.0, scale=rstd)
            nc.scalar.activation(or1, pp1, AF.Copy, bias=0.0, scale=rstd)
            ot0 = opool.tile([128, 512], BF16, tag="ot")
            ot1 = opool.tile([128, 512], BF16, tag="ot")
            nc.vector.tensor_sub(ot0, or0, t1[:, 0:512])
            nc.vector.tensor_sub(ot1, or1, t1[:, 512:1024])
            # spread the 2MB of output across all three DMA rings
            out_q = [nc.sync, nc.gpsimd, nc.scalar]
            out_q[(2 * nt) % 3].dma_start(
                out=out_d[nt * 128:(nt + 1) * 128, 0:512], in_=ot0)
            out_q[(2 * nt + 1) % 3].dma_start(
                out=out_d[nt * 128:(nt + 1) * 128, 512:1024], in_=ot1)

        # dummy sqrt queued right behind the last exps: ScalarE pays its
        # single table-set switch here, under pair 5's tails, instead of on
        # the first n-tile's rstd in the final phase
        warm_sq = tpool.tile([128, 1], F32, tag="rstd")
        nc.scalar.activation(warm_sq, eps_t, AF.Sqrt, bias=0.0, scale=1.0)
        # tail-only periods: pairs 5 and 6 (pair 6 also emits its own
        # transposes -- there is no later pair to lag them into)
        ep = pend.pop(0)
        emit_pair_tail(done, ep, alt=True)
        done += 1
        ep = pend.pop(0)
        emit_pair_tail(done, ep, alt=True, also_self_tr=True)
        done += 1
        # final pair: nt-major AV with the LN stats and the output
        # projection of earlier n-tiles interleaved, so the PE stays on
        # projection matmuls while the DVE runs stats chains.
        ep = pend.pop(0)
        gv0s_e = emit_gv0(2 * done)
        gv0s_o = emit_gv0(2 * done + 1)
        L_q = []
        for nt in range(NT):
            # pair 7's transposes; psum evacuations ride ScalarE here (the
            # exp stream is done)
            emit_av_nt(done, ep, nt, gv0s_e, gv0s_o, alt=False)
            emit_tr(done, nt, engine=nc.scalar)
            rstd, t1 = emit_stats(nt)
            L_q.append((nt, rstd, t1))
            # depth 1: nothing in the projection matmuls waits on rstd
            # anymore (the ScalarE evacuation does), and the 4-slot pp
            # rotation absorbs the evacuation lag -- a deeper queue only
            # lengthens the end-of-kernel flush
            if len(L_q) > 1:
                emit_outproj(*L_q.pop(0))
        for item in L_q:
            emit_outproj(*item)


def build_program(with_bias=False):
    key = ("nc", with_bias)
    if key in _CACHE:
        return _CACHE[key]
    nc = bacc.Bacc("TRN2", target_bir_lowering=False, debug=False, num_devices=8,
                   enable_partition_id=False)
    with tile.TileContext(nc) as tc:
        _emit(tc, with_bias)
    nc.compile()
    _CACHE[key] = nc
    return nc


def prep_inputs(x, x_text, Wq, Wk, Wv, gate, ln_g, ln_b, Wp, bp):
    """Host-side sharding/layout prep. Returns the 8 per-core input maps."""
    bf = ml_dtypes.bfloat16
    e4 = ml_dtypes.float8_e4m3
    x = np.asarray(x, np.float32)
    x_text = np.asarray(x_text, np.float32)
    xcat = np.concatenate([x_text, x], axis=1)          # [B, KV, DIM]
    xcatT = np.zeros((B, DIM, KVP), np.float32)
    xcatT[:, :, :KV] = xcat.transpose(0, 2, 1)
    xcat8 = np.clip(xcatT, -240, 240).astype(e4)
    xcatT = xcatT.astype(bf)
    wq8T = np.clip(np.asarray(Wq, np.float32).T * WSCALE, -240, 240).astype(e4)
    wk8T = np.clip(np.asarray(Wk, np.float32).T * WSCALE, -240, 240).astype(e4)
    wq8T = np.ascontiguousarray(wq8T)
    wk8T = np.ascontiguousarray(wk8T)
    wvT = np.ascontiguousarray(np.asarray(Wv, np.float32).T).astype(bf)
    # fold LayerNorm affine into the output projection:
    #   ((L - mu)*rstd*g + b) @ Wp.T + bp
    #     == (attn @ (Wp*g).T)*rstd - (mu*rstd*colsum(Wp*g) - (bp + Wp@b))
    Wp = np.asarray(Wp, np.float32)
    g = np.asarray(ln_g, np.float32).reshape(DIM)
    bvec = np.asarray(ln_b, np.float32).reshape(DIM)
    Wpf = Wp * g[None, :]
    bpf = np.asarray(bp, np.float32).reshape(DIM) + Wp @ bvec
    wpT = np.ascontiguousarray(Wpf.T).astype(bf)
    wbar = Wpf.sum(axis=1)                               # colsum over c, [DIM]
    wbarbp = np.stack([wbar, bpf]).astype(bf)            # [2, DIM]
    tanhg = np.tanh(np.asarray(gate, np.float32)).reshape(1, H).astype(np.float32)
    in_maps = []
    for b in range(B):
        in_maps.append({
            "xcat8": np.ascontiguousarray(xcat8[b]),
            "xcatT": np.ascontiguousarray(xcatT[b]),
            "wq8T": wq8T, "wk8T": wk8T, "wvT": wvT, "wpT": wpT,
            "tanhg": tanhg, "wbarbp": wbarbp,
        })
    return in_maps


def kernel(**inputs):
    global LAST_EXEC_NS
    in_maps = prep_inputs(**inputs)
    with_bias = bool(np.any(np.asarray(in_maps[0]["wbarbp"][1], np.float32)))
    nc = build_program(with_bias)
    trace = bool(int(os.environ.get("BASS_TRACE_RUN", "0")))
    res = run_bass_kernel_spmd(
        nc, in_maps, core_ids=list(range(8)), trace=trace,
    )
    LAST_EXEC_NS = res.exec_time_ns
    out = np.stack([r["out"] for r in res.results], axis=0)
    return out.astype(np.float32)


# revision 69
# speedup vs baseline: 1.2260x; 1.2260x over previous
"""Trainium2 Bass kernel for nn_Attention_45724221833663 (sparse_attention).

Strategy: data-parallel over batch B=8 across the 8 NeuronCores (one batch
element per core).  Matmul dtype mix: the q/k projections run in fp8e4
DoubleRow (2 contraction rows per PE cell -> half the matmul passes), the
softmax weights E are stored fp8 (AV = fp8 stationary x bf16 moving), and
everything whose error would reach the output directly (v projection,
scores operands, AV values, output projection) stays bf16 with fp32 PSUM.
Validated end-to-end: amax rel err ~8e-3 vs fp32 reference (tol 2e-2).

Per-core dataflow (host pre-transposes weights/x while sharding; ln_g is
folded into Wp, ln_b into bp):
  xcat8  [c=1024, kvp=1152] fp8  (concat(x_text,x).T, q/k weights x256 on
         host, the 1/65536 is folded into the exp scale)
  xcatT  [c, kvp] bf16           (v projection only)
  vw     [kvp, h, 65] bf16 = (xcatT.T @ WvT) per head + ones column
  qT/kT via DoubleRow fp8: 4 passes of K=256 instead of 8 of K=128
  per head pair (even head on PE row-tile 0, odd on row-tile 64):
    scores for both heads go into ONE [128,1024] psum tile per (kv-tile,
    n-half): disjoint PE row groups + disjoint psum banks -> the two
    matmuls execute concurrently.
    E = exp(scoresT/(8*65536)) -> fp8 e2 tile; kv=0 row and pad rows zeroed
    avp[n,0:65] = sum_kv E[kv,..] @ vw[kv,h,:]  (col 64 = S[n])
    attn[n, h*64:+64] = avp[:, :64]*(1/S) + tanh(g_h)*v_h[kv=0]
  Final head pair runs n-tile-major with LN stats + output projection
  interleaved.  The projection consumes RAW attn (transposed via PE), and
  LayerNorm is applied algebraically at psum evacuation:
      out = (attnT @ Wpf) * rstd - (mu*rstd*colsum(Wpf) - bp)
  so the PE transpose/matmul chain never waits on the LN stats.  rstd is
  computed as exp(-0.5*ln(var+eps)) so ScalarE stays on the
  natural_log_exp table set (no mid-kernel table switch).  Transposes are
  hoisted 2 chunks ahead of their matmuls and their psum->sbuf copies run
  on ScalarE (idle there: all exps are done by that phase).  Output is
  written bf16 and upcast on the host.
"""

import os
import numpy as np
import ml_dtypes

import concourse.bacc as bacc
import concourse.tile as tile
from concourse import mybir
from concourse.masks import make_identity
from concourse.bass_utils import run_bass_kernel_spmd

F32 = mybir.dt.float32
BF16 = mybir.dt.bfloat16
E4 = mybir.dt.float8e4
AF = mybir.ActivationFunctionType
OP = mybir.AluOpType
DRMODE = mybir.MatmulPerfMode.DoubleRow

B, N, P, DIM, H = 8, 1024, 77, 1024, 16
HD = DIM // H          # 64
KV = P + N             # 1101
KT = 9                 # kv tiles of 128
KVP = KT * 128         # 1152 padded
NT = N // 128          # 8 n tiles
CC = DIM // 128        # 8 contraction chunks
DR = CC // 2           # 4 double-row chunks (K=256 each)
OT = DIM // 128        # 8 output-channel tiles
LN_EPS = 1e-5
WSCALE = 256.0         # host premultiply on Wq/Wk so fp8 sees ~N(0,5) values
EXP_SCALE = 0.125 / (WSCALE * WSCALE)

LAST_EXEC_NS = None
_CACHE = {}


def _emit(tc, with_bias=False):
    nc = tc.nc

    xcat8_d = nc.dram_tensor("xcat8", [DIM, KVP], E4, kind="ExternalInput").ap()
    xcatT_d = nc.dram_tensor("xcatT", [DIM, KVP], BF16, kind="ExternalInput").ap()
    wq8_d = nc.dram_tensor("wq8T", [DIM, DIM], E4, kind="ExternalInput").ap()
    wk8_d = nc.dram_tensor("wk8T", [DIM, DIM], E4, kind="ExternalInput").ap()
    wv_d = nc.dram_tensor("wvT", [DIM, DIM], BF16, kind="ExternalInput").ap()
    wp_d = nc.dram_tensor("wpT", [DIM, DIM], BF16, kind="ExternalInput").ap()
    tanhg_d = nc.dram_tensor("tanhg", [1, H], F32, kind="ExternalInput").ap()
    wbarbp_d = nc.dram_tensor("wbarbp", [2, DIM], BF16, kind="ExternalInput").ap()
    out_d = nc.dram_tensor("out", [N, DIM], BF16, kind="ExternalOutput").ap()

    xcat8_re = xcat8_d.rearrange("(j p) f -> p j f", p=128)
    xcat_re = xcatT_d.rearrange("(j p) f -> p j f", p=128)
    wq8_re = wq8_d.rearrange("(j p) o -> p j o", p=128)
    wk8_re = wk8_d.rearrange("(j p) o -> p j o", p=128)
    wv_re = wv_d.rearrange("(j p) o -> p j o", p=128)
    wp_re = wp_d.rearrange("(j p) o -> p j o", p=128)

    from contextlib import ExitStack

    with ExitStack() as top:
        consts = top.enter_context(tc.tile_pool(name="consts", bufs=1))
        acts = top.enter_context(tc.tile_pool(name="acts", bufs=1))
        ph1 = top.enter_context(tc.tile_pool(name="ph1", bufs=1))
        wstream = top.enter_context(tc.tile_pool(name="wstream", bufs=8))
        qkp = top.enter_context(tc.tile_pool(name="qkp", bufs=3))
        epool = top.enter_context(tc.tile_pool(name="epool", bufs=4))
        tpool = top.enter_context(tc.tile_pool(name="tmp", bufs=4))
        opool = top.enter_context(tc.tile_pool(name="outp", bufs=4))
        t1p = top.enter_context(tc.tile_pool(name="t1p", bufs=2))
        ps_proj = top.enter_context(tc.tile_pool(name="ps_proj", bufs=2, space="PSUM"))
        ps_scores = top.enter_context(
            tc.tile_pool(name="ps_scores", bufs=2, space="PSUM"))
        ps_av = top.enter_context(tc.tile_pool(name="ps_av", bufs=2, space="PSUM"))

        # ---- constants ----
        tanhg_sb = consts.tile([128, H], F32, tag="tanhg")
        eps_t = consts.tile([128, 1], F32, tag="eps")
        nc.vector.memset(eps_t, LN_EPS)
        ident = consts.tile([128, 128], BF16, tag="ident")
        make_identity(nc, ident)
        wbar_b = consts.tile([128, DIM], BF16, tag="wbar")
        bp_b = consts.tile([128, DIM], BF16, tag="bpb")

        # p-state warmup: the PE idles ~5us waiting for the first input
        # chunks and then runs pair 0's projections at the cold 1.2GHz
        # p-state (it only reaches 2.4GHz after ~3.4us of continuous work).
        # Burn that idle window on zero matmuls sized to drain just before
        # the real ones are data-ready.
        warm = consts.tile([128, 512], BF16, tag="warm")
        nc.vector.memset(warm, 0.0)
        wps = ps_av.tile([128, 512], F32, tag="avp")
        for i in range(12):
            nc.tensor.matmul(wps, warm[:, 0:128], warm,
                             start=(i == 0), stop=(i == 11))

        def emit_fillers(n):
            # zero matmuls emitted AHEAD of a DMA-gated projection: they
            # execute while the projection's inputs are still in flight, so
            # the PE stays busy and HAM never re-throttles the clock.  Cost
            # if the data was actually ready: n x ~216ns.
            for i in range(n):
                nc.tensor.matmul(wps, warm[:, 0:128], warm,
                                 start=(i == 0), stop=(i == n - 1))

        # ---- persistent activations ----
        vw_sb = acts.tile([128, KT, H, HD + 1], BF16, tag="vw")  # [kv-part, kv-tile, h, d+1]
        attn_sb = acts.tile([128, NT, H, HD], BF16, tag="attn")  # [n-part, n-tile, h, d]
        # transposed attn chunks for the output projection.  Chunk cc of the
        # channel dim is exactly head pair cc, so each pair's transposes run
        # right after its AV tails -- spread across the whole kernel instead
        # of bunched into the final phase.
        lt_sb = acts.tile([128, NT, CC, 128], BF16, tag="lt")    # [c-part, n-tile, pair, 2*d]

        # input loads, c-chunk granular; fp8 xcat gates the q/k projections
        # (and hence scores + the exp stream) so it loads first; the bf16
        # xcat only feeds the v projection ~25us in.
        xcat8_sb = ph1.tile([128, CC, KVP], E4, tag="xcat8")
        xcatT_sb = ph1.tile([128, CC, KVP], BF16, tag="xcatT")
        # wv and wp share one slot: wv dies after the v projection, wp is
        # only needed from the output projection onwards
        wv_sb = ph1.tile([128, CC, DIM], BF16, tag="wvwp")
        # descriptor issue on an engine queue costs ~600ns each; spread the
        # startup-critical loads over the three DMA-capable queues.  Weight
        # streams for upcoming o-tiles are prefetched 2-3 pairs ahead so a
        # q/k projection never waits behind bulk xcat/wv traffic.
        w0q = wstream.tile([128, CC, 128], E4, tag="w")
        nc.scalar.dma_start(out=w0q, in_=wq8_re[:, :, 0:128])
        w0k = wstream.tile([128, CC, 128], E4, tag="w")
        nc.scalar.dma_start(out=w0k, in_=wk8_re[:, :, 0:128])

        wtiles = {}

        def prefetch_w(ot, q):
            if ot >= OT or ot in wtiles:
                return
            wq_t = wstream.tile([128, CC, 128], E4, tag="w", name=f"wq{ot}")
            q.dma_start(out=wq_t, in_=wq8_re[:, :, ot * 128:(ot + 1) * 128])
            wk_t = wstream.tile([128, CC, 128], E4, tag="w", name=f"wk{ot}")
            q.dma_start(out=wk_t, in_=wk8_re[:, :, ot * 128:(ot + 1) * 128])
            wtiles[ot] = (wq_t, wk_t)

        xcat_q = [nc.gpsimd, nc.gpsimd, nc.gpsimd, nc.gpsimd,
                  nc.scalar, nc.sync, nc.sync, nc.sync]
        # two pieces per chunk, all first pieces ahead of all second pieces:
        # cols 0:640 cover everything the q half-0 (cols 77:589) and the
        # k split-0 (cols 0:512) projections read, so their matmuls -- and
        # the first scores/exps -- start sooner than with whole-chunk loads
        for cc in range(CC):
            xcat_q[cc].dma_start(out=xcat8_sb[:, cc, 0:640],
                                 in_=xcat8_re[:, cc, 0:640])
        for cc in range(CC):
            xcat_q[cc].dma_start(out=xcat8_sb[:, cc, 640:KVP],
                                 in_=xcat8_re[:, cc, 640:KVP])
        # weights for pairs 1-3 on the (light) scalar ring, behind only the
        # two startup-critical xcat8 pieces it carries
        for ot in (1, 2, 3):
            prefetch_w(ot, nc.scalar)
        # bf16 xcat for the v projection: after the fp8 criticals
        xcatT_q = [nc.gpsimd, nc.gpsimd, nc.gpsimd, nc.gpsimd,
                   nc.sync, nc.sync, nc.sync, nc.sync]
        for cc in range(CC):
            xcatT_q[cc].dma_start(out=xcatT_sb[:, cc, :], in_=xcat_re[:, cc, :])

        # ---- q/k projections interleaved with their dependent head pairs,
        # so ScalarE (exp) fills while PE still runs projections ----
        last_rows = KV - (KT - 1) * 128  # 77

        def vproj_groups():
            # v projection into vw (head-interleaved), natural [kv, o]
            # layout, as 18 independent closures for interleaving
            def grp(kvt, half):
                def run():
                    ps = ps_proj.tile([128, 512], F32, tag="ps")
                    for cc in range(CC):
                        nc.tensor.matmul(
                            ps,
                            xcatT_sb[:, cc, kvt * 128:(kvt + 1) * 128],
                            wv_sb[:, cc, half * 512:(half + 1) * 512],
                            start=(cc == 0),
                            stop=(cc == CC - 1),
                        )
                    nc.vector.tensor_copy(
                        vw_sb[:, kvt, half * 8:(half + 1) * 8, 0:HD],
                        ps.rearrange("p (h d) -> p h d", d=HD),
                    )
                return run
            return [grp(kvt, half) for kvt in range(KT) for half in range(2)]

        def emit_qk_groups(ot, wtq=None, wtk=None):
            """q/k projections for o-tile ot as 5 closures (2 q halves,
            2 k splits, k text tail) so they can interleave between the
            scores chunks of the previous pair."""
            qt = qkp.tile([128, N], BF16, tag="qt")
            kt = qkp.tile([128, KVP], BF16, tag="kt")
            # pad keys (kv 1101:1152) are zero; scores psum partitions for
            # them are never read by the exp, but zero them for the checker
            nc.gpsimd.memset(kt[:, KV:KVP], 0.0)
            if wtq is None and ot in wtiles:
                wtq, wtk = wtiles.pop(ot)
            prefetch_w(ot + 2, nc.gpsimd)
            if wtq is None:
                wtq = wstream.tile([128, CC, 128], E4, tag="w")
                nc.sync.dma_start(out=wtq, in_=wq8_re[:, :, ot * 128:(ot + 1) * 128])
            if wtk is None:
                wtk = wstream.tile([128, CC, 128], E4, tag="w")
                nc.sync.dma_start(out=wtk, in_=wk8_re[:, :, ot * 128:(ot + 1) * 128])

            def qhalf(half):
                def run():
                    ps = ps_proj.tile([128, 512], F32, tag="ps")
                    for j in range(DR):
                        nc.tensor.matmul(
                            ps,
                            wtq[:, 2 * j:2 * j + 2, :],
                            xcat8_sb[:, 2 * j:2 * j + 2,
                                     P + half * 512: P + (half + 1) * 512],
                            start=(j == 0),
                            stop=(j == DR - 1),
                            perf_mode=DRMODE,
                        )
                    nc.vector.tensor_copy(qt[:, half * 512:(half + 1) * 512], ps)
                return run

            def ksplit(off):
                def run():
                    ps = ps_proj.tile([128, 512], F32, tag="ps")
                    for j in range(DR):
                        nc.tensor.matmul(
                            ps,
                            wtk[:, 2 * j:2 * j + 2, :],
                            xcat8_sb[:, 2 * j:2 * j + 2, off:off + 512],
                            start=(j == 0),
                            stop=(j == DR - 1),
                            perf_mode=DRMODE,
                        )
                    nc.vector.tensor_copy(kt[:, off:off + 512], ps)
                return run

            def k77():
                # 77-col text tail: DoubleRow loses below 128 free cols, so
                # run it as plain fp8 matmuls (bf16 rate)
                ps = ps_proj.tile([128, 512], F32, tag="ps")
                for cc in range(CC):
                    nc.tensor.matmul(
                        ps[:, :last_rows],
                        wtk[:, cc, :],
                        xcat8_sb[:, cc, 1024:KV],
                        start=(cc == 0),
                        stop=(cc == CC - 1),
                    )
                nc.vector.tensor_copy(kt[:, 1024:KV], ps[:, :last_rows])

            return qt, kt, [qhalf(0), ksplit(0), qhalf(1), ksplit(512), k77]

        def new_e2():
            # E layout: [kv-part, kv-tile, n-half, head, 512], fp8
            e2 = epool.tile([128, KT, 2, 2, 512], E4, tag="e")
            nc.gpsimd.memset(e2[:, KT - 1], 0.0)
            return e2

        def emit_scores_pair(qt, kt):
            # Scores for the even/odd head pair.  Both heads of a (kv-tile,
            # n-half) share ONE [128,1024] psum tile: even head -> cols
            # 0:512 on PE row-tile 0, odd head -> cols 512:1024 on row-tile
            # 64.  Disjoint row groups + disjoint psum banks mean the two
            # matmuls execute concurrently.  Half-major order so the AV of
            # n-tiles 0-3 (which only needs half 0) starts as soon as half
            # 0's exps drain.  (A 2048-col single-slot variant halves the
            # exp instruction overhead but its psum WAR convoy costs far
            # more -- measured twice.)
            e2 = new_e2()
            for half in range(2):
                for kvt in range(KT):
                    rows = last_rows if kvt == KT - 1 else 128
                    ps = ps_scores.tile([128, 1024], F32, tag="pss")
                    nc.tensor.matmul(
                        ps[:, 0:512],
                        kt[0:64, kvt * 128:(kvt + 1) * 128],
                        qt[0:64, half * 512:(half + 1) * 512],
                        start=True, stop=True,
                    )
                    nc.tensor.matmul(
                        ps[:, 512:1024],
                        kt[64:128, kvt * 128:(kvt + 1) * 128],
                        qt[64:128, half * 512:(half + 1) * 512],
                        start=True, stop=True,
                    )
                    nc.scalar.activation(
                        e2[:rows, kvt, half], ps[:rows], AF.Exp,
                        bias=0.0, scale=EXP_SCALE)
            # first key column is gated separately; one memset per n-half so
            # the AV of n-tiles 0-3 only depends on half 0's exps
            nc.gpsimd.memset(e2[0:1, 0, 0], 0.0)
            nc.gpsimd.memset(e2[0:1, 0, 1], 0.0)
            return e2

        def eslice(e2, hh, kvt, nt):
            q, r = divmod(nt, 4)
            return e2[:, kvt, q, hh, r * 128:(r + 1) * 128]

        def emit_gv0(h):
            gv0 = tpool.tile([128, HD], BF16, tag="gv0")
            nc.gpsimd.partition_broadcast(gv0, vw_sb[0:1, 0, h, 0:HD])
            gv0s = tpool.tile([128, HD], F32, tag="gv0s")
            nc.vector.tensor_scalar_mul(gv0s, gv0, tanhg_sb[:, h:h + 1])
            return gv0s

        def emit_tr(p, nt, engine=None):
            """Transpose attn heads (2p, 2p+1) at n-tile nt into LT --
            chunk p of the output projection's lhsT.  Interleaved into the
            pair's AV loop, spreading the PE transposes and their psum
            evacuations (DVE mid-kernel -- gpsimd cannot read PSUM on trn2;
            ScalarE in the final phase where the exp stream is done)."""
            pst = ps_av.tile([128, 128], BF16, tag="avp")
            nc.tensor.transpose(
                pst,
                attn_sb[:, nt, 2 * p:2 * p + 2, :].rearrange("p h d -> p (h d)"),
                ident,
            )
            if engine is None:
                nc.vector.tensor_copy(lt_sb[:, nt, p, :], pst)
            else:
                engine.copy(out=lt_sb[:, nt, p, :], in_=pst)

        def emit_av_nt(p, e2, nt, gv0s_e, gv0s_o, alt, tr_p=None):
            """AV + fixup for BOTH heads of pair p at n-tile nt, plus the
            LT transpose of pair tr_p (lagged one pair behind, so its attn
            input's DVE fixups are long done and the PE never waits).
            Both heads accumulate into one [128,2,65] psum tile (one
            bank), so the pair costs one psum slot turn and one reciprocal
            instead of two -- the DVE's per-instruction fixed cost was the
            hidden hog at [128,64] granularity."""
            if alt and nt % 2 == 1:
                avp = ps_proj.tile([128, 2, HD + 1], F32, tag="ps")
            else:
                avp = ps_av.tile([128, 2, HD + 1], F32, tag="avp")
            for hh in range(2):
                for kvt in range(KT):
                    nc.tensor.matmul(
                        avp[:, hh, :],
                        eslice(e2, hh, kvt, nt),
                        vw_sb[:, kvt, 2 * p + hh, :],
                        start=(kvt == 0),
                        stop=(kvt == KT - 1),
                    )
            rs2 = tpool.tile([128, 2], F32, tag="rs")
            nc.vector.reciprocal(rs2, avp[:, :, HD])
            nc.vector.scalar_tensor_tensor(
                out=attn_sb[:, nt, 2 * p, :],
                in0=avp[:, 0, 0:HD],
                scalar=rs2[:, 0:1],
                in1=gv0s_e,
                op0=OP.mult,
                op1=OP.add,
            )
            nc.vector.scalar_tensor_tensor(
                out=attn_sb[:, nt, 2 * p + 1, :],
                in0=avp[:, 1, 0:HD],
                scalar=rs2[:, 1:2],
                in1=gv0s_o,
                op0=OP.mult,
                op1=OP.add,
            )
            if tr_p is not None:
                emit_tr(tr_p, nt)

        def emit_pair_tail(p, e2, alt=False, also_self_tr=False):
            """Everything after E for pair p: gate prep, AV + fixup, plus
            the LAGGED LT transposes of pair p-1 (and p's own when this is
            the last non-final tail).  alt=True additionally cycles the
            (by-then idle) proj psum pool for deeper AV pipelining."""
            gv0s_e = emit_gv0(2 * p)
            gv0s_o = emit_gv0(2 * p + 1)
            for nt in range(NT):
                emit_av_nt(p, e2, nt, gv0s_e, gv0s_o, alt,
                           tr_p=(p - 1 if p >= 1 else None))
                if also_self_tr:
                    emit_tr(p, nt)

        # ---- LN stats per n-tile.  Only mu/rstd are computed here; the
        # normalization itself is folded into the projection's psum
        # evacuation, so the PE transposes/matmuls never wait on it. ----
        def emit_stats(nt):
            xa = attn_sb[:, nt].rearrange("p h d -> p (h d)")
            xs = xa.rearrange("p (s f) -> p s f", f=512)
            stats = tpool.tile([128, 2, 6], F32, tag="stats")
            for s in range(2):
                nc.vector.bn_stats(stats[:, s, :], xs[:, s, :])
            mv = tpool.tile([128, 2], F32, tag="mv")
            nc.vector.bn_aggr(mv, stats)
            # sqrt + DVE reciprocal.  All sqrts happen after the last exp,
            # so ScalarE switches table sets exactly once (a dummy sqrt
            # right after the last scores pair pre-pays the ~1.3us load);
            # Copy lives in every set, so the psum evacuations never force
            # a reload.  (Ln/Exp rstd alternated table sets PER N-TILE --
            # ~1.3us reload each way, caught in the trace.)
            rstd = tpool.tile([128, 1], F32, tag="rstd")
            nc.scalar.activation(rstd, mv[:, 1:2], AF.Sqrt, bias=eps_t, scale=1.0)
            nc.vector.reciprocal(rstd, rstd)
            m2 = tpool.tile([128, 1], F32, tag="m2")
            nc.vector.tensor_scalar_mul(m2, mv[:, 0:1], rstd)
            # t1[n, o] = mu*rstd*colsum(Wpf)[o] (- bp[o] when the folded
            # bias is nonzero; it is zero here so the fast path uses a
            # cheap 2-operand multiply)
            t1 = t1p.tile([128, DIM], BF16, tag="t1")
            if with_bias:
                nc.vector.scalar_tensor_tensor(
                    out=t1, in0=wbar_b, scalar=m2, in1=bp_b,
                    op0=OP.mult, op1=OP.subtract,
                )
            else:
                nc.vector.tensor_scalar_mul(t1, wbar_b, m2)
            return rstd, t1

        # pair 0's scores/exp are hoisted before the v projection so ScalarE
        # starts as early as possible
        # software pipeline: scores/exp run one head-pair ahead of the
        # AV/fixup tails so ScalarE never starves
        # ---- main flow (half-major scores, 2-pair software pipeline) ----
        qt_c, kt_c, g0 = emit_qk_groups(0, w0q, w0k)
        for g in g0:
            g()
        # wv split sync/scalar: one queue alone delivers the last chunk too
        # late for the v-projection (keep gpsimd free for the e2 memsets;
        # the scalar queue's exp stream only starts at the first pair)
        for cc in range(CC):
            dmae = nc.sync if cc % 2 == 0 else nc.scalar
            dmae.dma_start(out=wv_sb[:, cc, :], in_=wv_re[:, cc, :])
        pend = [emit_scores_pair(qt_c, kt_c)]
        emit_fillers(6)
        qt_c, kt_c, gq = emit_qk_groups(1)
        for g in gq:
            g()
        pend.append(emit_scores_pair(qt_c, kt_c))
        # ones column for the row-sum S (E rows for kv=0/pad are zeroed);
        # disjoint from the v-projection's columns, so set it up front
        nc.gpsimd.memset(vw_sb[:, :, :, HD:HD + 1], 1.0)
        # tanh(gate): first consumer is the head tails ~45us in, keep it off
        # the startup-critical queues
        nc.sync.dma_start(out=tanhg_sb, in_=tanhg_d.to_broadcast([128, H]))
        # pairs 2-3's scores BEFORE the v-projection: four banked pairs of
        # exp work keep ScalarE fed through it and give the bf16-xcat/wv
        # DMAs time to land
        vgs = vproj_groups()
        emit_fillers(4)
        qt_c, kt_c, gq = emit_qk_groups(2)
        for g in gq:
            g()
        pend.append(emit_scores_pair(qt_c, kt_c))
        # first v-projection tiles between the early scores pairs: the
        # exp-independent v matmuls keep the PE busy while pairs 0-1's exps
        # drain the scores psum
        for g in vgs[:6]:
            g()
        qt_c, kt_c, gq = emit_qk_groups(3)
        for g in gq:
            g()
        pend.append(emit_scores_pair(qt_c, kt_c))
        for g in vgs[6:]:
            g()
        wp_sb = ph1.tile([128, CC, DIM], BF16, tag="wvwp")
        for cc in range(CC):
            dmae = nc.gpsimd if cc < 4 else nc.sync
            dmae.dma_start(out=wp_sb[:, cc, :], in_=wp_re[:, cc, :])
        # folded colsum(Wpf)/bp broadcasts: consumed from ~200us
        nc.sync.dma_start(out=wbar_b, in_=wbarbp_d[0:1, :].to_broadcast([128, DIM]))
        nc.sync.dma_start(out=bp_b, in_=wbarbp_d[1:2, :].to_broadcast([128, DIM]))
        ep = pend.pop(0)
        emit_pair_tail(0, ep)
        ep = pend.pop(0)
        emit_pair_tail(1, ep)
        done = 2
        for ot in range(4, OT):
            qt_c, kt_c, gq = emit_qk_groups(ot)
            for g in gq:
                g()
            pend.append(emit_scores_pair(qt_c, kt_c))
            ep = pend.pop(0)
            emit_pair_tail(done, ep, alt=(ot >= OT - 2))
            done += 1

        def emit_outproj(nt, rstd, t1):
            # project the transposed raw-attn chunks (banked in LT across
            # the whole kernel): out[n, o] = (attn @ Wpf.T)*rstd - t1
            # pp psum alternates between the proj pool and the (dead by
            # now) scores pool, giving a 4-slot rotation: the projection of
            # nt+1 never WAR-waits on nt's psum evacuation.
            if nt % 2 == 0:
                pp0 = ps_proj.tile([128, 512], F32, tag="ps")
                pp1 = ps_proj.tile([128, 512], F32, tag="ps")
            else:
                pp0 = ps_scores.tile([128, 512], F32, tag="pss")
                pp1 = ps_scores.tile([128, 512], F32, tag="pss")
            for cc in range(CC):
                nc.tensor.matmul(
                    pp0, lt_sb[:, nt, cc, :], wp_sb[:, cc, 0:512],
                    start=(cc == 0), stop=(cc == CC - 1),
                )
                nc.tensor.matmul(
                    pp1, lt_sb[:, nt, cc, :], wp_sb[:, cc, 512:1024],
                    start=(cc == 0), stop=(cc == CC - 1),
                )
            # evacuation with the rstd scale fused into the ScalarE psum
            # copy (free: the scale slot of ACTIVATE Copy takes a
            # per-partition AP); ScalarE is idle in this phase once the exp
            # backlog drains, while the DVE carries stats + fixups.  The
            # mean correction is then a cheap 2-operand subtract.
            or0 = opool.tile([128, 512], BF16, tag="or")
            or1 = opool.tile([128, 512], BF16, tag="or")
            nc.scalar.activation(or0, pp0, AF.Copy, bias=0.0, scale=rstd)
            nc.scalar.activation(or1, pp1, AF.Copy, bias=0.0, scale=rstd)
            ot0 = opool.tile([128, 512], BF16, tag="ot")
            ot1 = opool.tile([128, 512], BF16, tag="ot")
            nc.vector.tensor_sub(ot0, or0, t1[:, 0:512])
            nc.vector.tensor_sub(ot1, or1, t1[:, 512:1024])
            # spread the 2MB of output across all three DMA rings
            out_q = [nc.sync, nc.gpsimd, nc.scalar]
            out_q[(2 * nt) % 3].dma_start(
                out=out_d[nt * 128:(nt + 1) * 128, 0:512], in_=ot0)
            out_q[(2 * nt + 1) % 3].dma_start(
                out=out_d[nt * 128:(nt + 1) * 128, 512:1024], in_=ot1)

        # dummy sqrt queued right behind the last exps: ScalarE pays its
        # single table-set switch here, under pair 6's tails, instead of on
        # the first n-tile's rstd in the final phase
        warm_sq = tpool.tile([128, 1], F32, tag="rstd")
        nc.scalar.activation(warm_sq, eps_t, AF.Sqrt, bias=0.0, scale=1.0)
        ep = pend.pop(0)
        emit_pair_tail(done, ep, alt=True)
        done += 1
        # final pair: nt-major AV with the LN stats and the output
        # projection of earlier n-tiles interleaved, so the PE stays on
        # projection matmuls while the DVE runs stats chains.
        ep = pend.pop(0)
        gv0s_e = emit_gv0(2 * done)
        gv0s_o = emit_gv0(2 * done + 1)
        L_q = []
        for nt in range(NT):
            # lagged transposes of pair 6 plus pair 7's own (no next pair);
            # psum evacuations ride ScalarE here (the exp stream is done)
            emit_av_nt(done, ep, nt, gv0s_e, gv0s_o, alt=False)
            emit_tr(done - 1, nt, engine=nc.scalar)
            emit_tr(done, nt, engine=nc.scalar)
            rstd, t1 = emit_stats(nt)
            L_q.append((nt, rstd, t1))
            # depth 1: nothing in the projection matmuls waits on rstd
            # anymore (the ScalarE evacuation does), and the 4-slot pp
            # rotation absorbs the evacuation lag -- a deeper queue only
            # lengthens the end-of-kernel flush
            if len(L_q) > 1:
                emit_outproj(*L_q.pop(0))
        for item in L_q:
            emit_outproj(*item)


def build_program(with_bias=False):
    key = ("nc", with_bias)
    if key in _CACHE:
        return _CACHE[key]
    nc = bacc.Bacc("TRN2", target_bir_lowering=False, debug=False, num_devices=8,
                   enable_partition_id=False)
    with tile.TileContext(nc) as tc:
        _emit(tc, with_bias)
    nc.compile()
    _CACHE[key] = nc
    return nc


def prep_inputs(x, x_text, Wq, Wk, Wv, gate, ln_g, ln_b, Wp, bp):
    """Host-side sharding/layout prep. Returns the 8 per-core input maps."""
    bf = ml_dtypes.bfloat16
    e4 = ml_dtypes.float8_e4m3
    x = np.asarray(x, np.float32)
    x_text = np.asarray(x_text, np.float32)
    xcat = np.concatenate([x_text, x], axis=1)          # [B, KV, DIM]
    xcatT = np.zeros((B, DIM, KVP), np.float32)
    xcatT[:, :, :KV] = xcat.transpose(0, 2, 1)
    xcat8 = np.clip(xcatT, -240, 240).astype(e4)
    xcatT = xcatT.astype(bf)
    wq8T = np.clip(np.asarray(Wq, np.float32).T * WSCALE, -240, 240).astype(e4)
    wk8T = np.clip(np.asarray(Wk, np.float32).T * WSCALE, -240, 240).astype(e4)
    wq8T = np.ascontiguousarray(wq8T)
    wk8T = np.ascontiguousarray(wk8T)
    wvT = np.ascontiguousarray(np.asarray(Wv, np.float32).T).astype(bf)
    # fold LayerNorm affine into the output projection:
    #   ((L - mu)*rstd*g + b) @ Wp.T + bp
    #     == (attn @ (Wp*g).T)*rstd - (mu*rstd*colsum(Wp*g) - (bp + Wp@b))
    Wp = np.asarray(Wp, np.float32)
    g = np.asarray(ln_g, np.float32).reshape(DIM)
    bvec = np.asarray(ln_b, np.float32).reshape(DIM)
    Wpf = Wp * g[None, :]
    bpf = np.asarray(bp, np.float32).reshape(DIM) + Wp @ bvec
    wpT = np.ascontiguousarray(Wpf.T).astype(bf)
    wbar = Wpf.sum(axis=1)                               # colsum over c, [DIM]
    wbarbp = np.stack([wbar, bpf]).astype(bf)            # [2, DIM]
    tanhg = np.tanh(np.asarray(gate, np.float32)).reshape(1, H).astype(np.float32)
    in_maps = []
    for b in range(B):
        in_maps.append({
            "xcat8": np.ascontiguousarray(xcat8[b]),
            "xcatT": np.ascontiguousarray(xcatT[b]),
            "wq8T": wq8T, "wk8T": wk8T, "wvT": wvT, "wpT": wpT,
            "tanhg": tanhg, "wbarbp": wbarbp,
        })
    return in_maps


def kernel(**inputs):
    global LAST_EXEC_NS
    in_maps = prep_inputs(**inputs)
    with_bias = bool(np.any(np.asarray(in_maps[0]["wbarbp"][1], np.float32)))
    nc = build_program(with_bias)
    trace = bool(int(os.environ.get("BASS_TRACE_RUN", "0")))
    res = run_bass_kernel_spmd(
        nc, in_maps, core_ids=list(range(8)), trace=trace,
    )
    LAST_EXEC_NS = res.exec_time_ns
    out = np.stack([r["out"] for r in res.results], axis=0)
    return out.astype(np.float32)
